# revision 17
# baseline (speedup 1.0000x reference)
"""Binarized 3x3 conv block on 8 Trainium2 NeuronCores — 1D-Winograd F(2,3).

Over the previous baseline (host-exact BN mean + two stat AllGathers):
- Per-device BN variance (sanctioned by the sharding hint): each core
  normalizes with var = E_local[y^2] - mu_global^2, where mu_global is the
  exact host-computed conv mean (linear in x) and E_local[y^2] averages the
  core's own 4 images. Validated against the reference in fp64: rel err
  6.9e-3 from the stats alone (tolerance 2e-2); the device bf16 error adds
  ~3.7e-3 in quadrature. This removes BOTH AllGathers, the sacrificial
  warm-up collective, the gather readback/transpose, and — critically — the
  inter-core skew coupling: each core's exec time is now its own span.
- BN apply factored as res = Relu(inv*q + beta) with q = gamma*(pmax - mu)
  precomputed per image DURING the conv (q is independent of the variance),
  so the post-stats critical path is reduce -> var -> Sqrt -> recip -> one
  or two ops per image.
- The last block of ch1 reconstructs its Winograd products directly from
  PSUM on the Vector engine (no serialized Scalar copies on the tail).
- ch0's epilogue keeps the Scalar queue clean (only the Sqrt) so ch1's
  first evictions never stall PSUM recycling at the chunk boundary.
- Prologue: weight DMA split per (cic, out-channel-half); the ch1 halves
  and stat vectors load after img0's x so the first matmul gates on ~1.3MB
  of HBM traffic; img0's x loads + width transforms run in 3 row-chunks
  matched to row-block consumption (rows 0-16 / 16-30 / 30-58).
- Outputs are stored as bf16 (upcast to f32 on the host during the gather).
- Fast path assumes gamma >= 0 (true for the shipped inputs; a general
  variant with the min-pool trick compiles on demand otherwise): maxpool
  commutes with the monotone BN apply.
"""

import numpy as np
import ml_dtypes

_NCORES = 8
_B, _C, _H, _W = 32, 256, 56, 56
_BS = _B // _NCORES          # images per core
_PH, _PW = _H + 2, _W + 2    # padded input
_OH, _OW = _H // 2, _W // 2  # pooled output
_EPS = 1e-5
_NSTAT_LOC = float(_BS * _H * _W)  # per-core elements per channel in stats
_BF16 = ml_dtypes.bfloat16

_CACHE: dict = {}


def _build(general: bool):
    import concourse.bacc as bacc
    import concourse.mybir as mybir
    import concourse.tile as tile

    f32 = mybir.dt.float32
    bf16 = mybir.dt.bfloat16
    AF = mybir.ActivationFunctionType
    AX = mybir.AxisListType
    OP = mybir.AluOpType

    nc = bacc.Bacc("TRN2", target_bir_lowering=False, debug=False,
                   num_devices=_NCORES)
    xp_d = nc.dram_tensor("xp", [_BS, _C, 2, _PH, _PW // 2], bf16,
                          kind="ExternalInput")
    w_d = nc.dram_tensor("wt", [2, 2, 128, 12, 128], bf16,
                         kind="ExternalInput")
    g_d = nc.dram_tensor("gm", [128, 2], f32, kind="ExternalInput")
    bt_d = nc.dram_tensor("bt", [128, 2], f32, kind="ExternalInput")
    mu_d = nc.dram_tensor("mu", [128, 2], f32, kind="ExternalInput")
    ngmu_d = nc.dram_tensor("ngmu", [128, 2], f32, kind="ExternalInput")
    nm2_d = nc.dram_tensor("nm2", [128, 2], f32, kind="ExternalInput")
    out_d = nc.dram_tensor("out", [_BS, _C, _OH, _OW], bf16,
                           kind="ExternalOutput")

    with tile.TileContext(nc) as tc:
        with (
            tc.tile_pool(name="persist", bufs=1) as keep,
            tc.tile_pool(name="xload", bufs=2) as xpool,
            tc.tile_pool(name="evict", bufs=3) as evp,
            tc.tile_pool(name="apply", bufs=4) as app,
            tc.tile_pool(name="acc", bufs=8, space="PSUM") as psp,
        ):
            # weights split per (cic, out-channel half): the first matmul
            # gates on the two ch0 halves only
            w_sb = [[keep.tile([128, 12, 128], bf16, tag=f"w{c}_{h}",
                               name=f"w{c}_{h}") for h in range(2)]
                    for c in range(2)]
            gm_sb = keep.tile([128, 2], f32, tag="gm", name="gm")
            bt_sb = keep.tile([128, 2], f32, tag="bt", name="bt")
            mu_sb = keep.tile([128, 2], f32, tag="mu", name="mu")
            ngmu_sb = keep.tile([128, 2], f32, tag="ngmu", name="ngmu")
            nm2_sb = keep.tile([128, 2], f32, tag="nm2", name="nm2")
            rdump = keep.tile([128, 4 * _BS + 1], f32, tag="rdump",
                              name="rdump")
            eps = keep.tile([128, 1], f32, tag="eps", name="eps")
            nc.gpsimd.memset(eps[:], _EPS)
            warm = keep.tile([128, 1], f32, tag="warm", name="warm")

            # one sum-of-squares column per (img, rb); ch1's tail block is
            # evicted in two halves, so it gets one extra column
            sqc = [keep.tile([128, 4 * _BS + 1], f32, tag=f"sq{c}",
                             name=f"sq{c}") for c in range(2)]
            pmax = [[keep.tile([128, _OH, _OW], bf16, tag=f"pmax{i}_{c}",
                               name=f"pmax{i}_{c}") for c in range(2)]
                    for i in range(_BS)]
            qt = [[keep.tile([128, _OH, _OW], bf16, tag=f"q{i}_{c}",
                             name=f"q{i}_{c}") for c in range(2)]
                  for i in range(_BS)]
            if general:
                pmin = [[keep.tile([128, _OH, _OW], bf16, tag=f"pmin{i}_{c}",
                                   name=f"pmin{i}_{c}") for c in range(2)]
                        for i in range(_BS)]
            gsq = [keep.tile([128, 1], f32, tag=f"gsq{c}", name=f"gsq{c}")
                   for c in range(2)]

            # ---- width-axis input transforms, kept resident for both chunks
            # V0 = d0-d2, V1 = d1+d2, V2 = d2-d1, V3 = d1-d3 where
            # d0,d2 = adjacent even cols and d1,d3 = adjacent odd cols;
            # the host ships x as even/odd planes so every read is stride-1
            vt = [[None] * 2 for _ in range(_BS)]

            def emit_transforms(img, eng=None):
                xs = []
                for cic in range(2):
                    vt[img][cic] = [keep.tile([128, _PH, _OW], bf16,
                                              tag=f"v{img}_{cic}_{l}",
                                              name=f"v{img}_{cic}_{l}")
                                    for l in range(4)]
                    xs.append(xpool.tile([128, 2, _PH, _PW // 2], bf16,
                                         tag=f"x{cic}",
                                         name=f"x{img}_{cic}"))
                # row-chunked loads matched to row-block consumption: rb0
                # needs vt rows 0-15, rb1 rows 14-29, rb2/3 the rest; img0
                # gets 3 chunks so its first matmul gates on ~0.5MB of x;
                # img2/3 (emitted mid-conv) use one full-height op per
                # plane — fewer DVE cycles in the congested stretch
                if img == 0:
                    chunks = ((0, 16), (16, 30), (30, _PH))
                elif img == 1:
                    chunks = ((0, 29), (29, _PH))
                else:
                    chunks = ((0, _PH),)
                for r0, r1 in chunks:
                    for cic in range(2):
                        nc.sync.dma_start(
                            xs[cic][:, :, r0:r1],
                            xp_d[img, cic * 128:(cic + 1) * 128, :, r0:r1])
                if eng is None:
                    eng = nc.vector
                # chunk-outer, l-major emission: short vector-queue blocks
                # (evictions interleave without stalling PSUM recycling) and
                # rb0's matmuls start after the first chunk's 8 small ops
                for r0, r1 in chunks:
                    for l in range(4):
                        for cic in range(2):
                            xe = xs[cic][:, 0, r0:r1]
                            xo = xs[cic][:, 1, r0:r1]
                            dst = vt[img][cic][l][:, r0:r1]
                            if l == 0:
                                eng.tensor_sub(dst, xe[:, :, 0:_OW],
                                               xe[:, :, 1:_OW + 1])
                            elif l == 1:
                                eng.tensor_add(dst, xo[:, :, 0:_OW],
                                               xe[:, :, 1:_OW + 1])
                            elif l == 2:
                                eng.tensor_sub(dst, xe[:, :, 1:_OW + 1],
                                               xo[:, :, 0:_OW])
                            else:
                                eng.tensor_sub(dst, xo[:, :, 0:_OW],
                                               xo[:, :, 1:_OW + 1])

            # weights lead the scalar queue (they gate the first matmul);
            # the ch0 halves load in l-granular row-triples so the first
            # matmul gates on ~196KB, and the ch1 halves + stat vectors
            # queue after img0's x traffic
            for l in range(4):
                nc.scalar.dma_start(w_sb[0][0][:, l * 3:(l + 1) * 3],
                                    w_d[0, 0, :, l * 3:(l + 1) * 3])
                nc.scalar.dma_start(w_sb[1][0][:, l * 3:(l + 1) * 3],
                                    w_d[1, 0, :, l * 3:(l + 1) * 3])
            emit_transforms(0)
            nc.scalar.dma_start(w_sb[0][1][:], w_d[0, 1])
            nc.scalar.dma_start(w_sb[1][1][:], w_d[1, 1])
            nc.scalar.dma_start(gm_sb[:], g_d[:])
            nc.scalar.dma_start(bt_sb[:], bt_d[:])
            nc.scalar.dma_start(mu_sb[:], mu_d[:])
            nc.scalar.dma_start(ngmu_sb[:], ngmu_d[:])
            nc.scalar.dma_start(nm2_sb[:], nm2_d[:])
            emit_transforms(1)
            # prologue dummy Sqrt: pulls the sqrt-set ACT_TABLE_LOAD off
            # the epilogue scale chain into the idle kernel start
            nc.scalar.activation(warm[:], eps[:], AF.Sqrt, bias=0.0)

            def emit_q(ch, img):
                # q = gamma*(pmax - mu); independent of the variance, so it
                # runs during the conv and the post-stats apply is tiny
                if general:
                    qx = app.tile([128, _OH, _OW], bf16, tag="qx",
                                  name=f"qx{ch}_{img}")
                    qn = app.tile([128, _OH, _OW], bf16, tag="qn",
                                  name=f"qn{ch}_{img}")
                    nc.vector.tensor_scalar(qx[:], pmax[img][ch][:],
                                            gm_sb[:, ch:ch + 1],
                                            ngmu_sb[:, ch:ch + 1],
                                            op0=OP.mult, op1=OP.add)
                    nc.vector.tensor_scalar(qn[:], pmin[img][ch][:],
                                            gm_sb[:, ch:ch + 1],
                                            ngmu_sb[:, ch:ch + 1],
                                            op0=OP.mult, op1=OP.add)
                    nc.vector.tensor_max(qt[img][ch][:], qx[:], qn[:])
                else:
                    nc.vector.tensor_scalar(qt[img][ch][:],
                                            pmax[img][ch][:],
                                            gm_sb[:, ch:ch + 1],
                                            ngmu_sb[:, ch:ch + 1],
                                            op0=OP.mult, op1=OP.add)

            # ---- conv + fused eviction + per-chunk epilogue ----
            # 4 row-blocks of 14 output rows; each Winograd product gets
            # its own single-bank PSUM tile so readers gate on just that
            # product's 6 matmuls
            pending_sq = []  # deferred Square emissions (see below)

            def flush_sq():
                # squares are emitted one block LATE so a vector-gated
                # Square never sits ahead of the PSUM-freeing evictions in
                # the Scalar FIFO
                while pending_sq:
                    src, colap = pending_sq.pop(0)
                    nc.scalar.activation(src[0], src[1], AF.Square,
                                         accum_out=colap)

            for ch in range(2):
                for img in range(_BS):
                    for rb in range(4):
                        pss = []
                        for l in range(4):
                            ps = psp.tile([128, 512], f32, tag="acc",
                                          name=f"acc{ch}_{img}_{rb}_{l}")
                            pss.append(ps)
                            # zero-row trim: (rb0, kh0) covers padded row 0
                            # and (rb3, kh2) padded row 57 — both all-zero.
                            # kh order keeps the start=True matmul full.
                            khs = (1, 0, 2) if rb == 0 else (0, 1, 2)
                            k = 0
                            for cic in range(2):
                                for kh in khs:
                                    r0 = rb * 14 + kh
                                    r1 = r0 + 14
                                    c0 = 0
                                    if rb == 0 and kh == 0:
                                        r0, c0 = 1, _OW
                                    elif rb == 3 and kh == 2:
                                        r1 = 57
                                    nc.tensor.matmul(
                                        ps[:, c0:(r1 - r0) * _OW + c0],
                                        w_sb[cic][ch][:, l * 3 + kh],
                                        vt[img][cic][l][:, r0:r1, :],
                                        start=(k == 0), stop=(k == 5))
                                    k += 1
                        col = img * 4 + rb
                        last_blk = (ch == 1 and img == _BS - 1 and rb == 3)
                        yeo = evp.tile([128, 2, 14, _OW], bf16, tag="yeo",
                                       name=f"yeo{ch}_{img}_{rb}")
                        t01 = evp.tile([128, 14, _OW], bf16, tag="t01",
                                       name=f"t01_{ch}_{img}_{rb}")
                        t12 = evp.tile([128, 14, _OW], bf16, tag="t12",
                                       name=f"t12_{ch}_{img}_{rb}")
                        t1 = evp.tile([128, 7, _OW], bf16, tag="t1",
                                      name=f"t1_{ch}_{img}_{rb}")
                        t2 = evp.tile([128, 7, _OW], bf16, tag="t2",
                                      name=f"t2_{ch}_{img}_{rb}")
                        sq1 = evp.tile([128, 2, 14, _OW], bf16, tag="sq1",
                                       name=f"sq1_{ch}_{img}_{rb}")
                        if last_blk and not general:
                            # tail block: per-product PSUM tiles let the
                            # reconstruction pre-run product by product (a
                            # DVE op may read only ONE PSUM operand, so M1
                            # is staged to SBUF by the Scalar engine);
                            # after the last matmul only yod, its square,
                            # and the odd pool precede the stats chain
                            flush_sq()
                            c1 = evp.tile([128, 14, _OW], bf16, tag="c1",
                                          name=f"c1_{ch}_{img}_{rb}")
                            nc.scalar.activation(c1[:], pss[1][:, 0:392],
                                                 AF.Copy)
                            nc.vector.tensor_add(t01[:], pss[0][:, 0:392],
                                                 c1[:])
                            nc.vector.tensor_sub(t12[:], c1[:],
                                                 pss[2][:, 0:392])
                            nc.vector.tensor_add(yeo[:, 0], t01[:],
                                                 pss[2][:, 0:392])
                            nc.scalar.activation(
                                sq1[:, 0], yeo[:, 0], AF.Square,
                                accum_out=sqc[ch][:, col:col + 1])
                            nc.vector.tensor_max(t1[:],
                                                 yeo[:, 0, 0:14:2, :],
                                                 yeo[:, 0, 1:14:2, :])
                            nc.vector.tensor_sub(yeo[:, 1], t12[:],
                                                 pss[3][:, 0:392])
                            nc.scalar.activation(
                                sq1[:, 1], yeo[:, 1], AF.Square,
                                accum_out=sqc[ch][:, col + 1:col + 2])
                            nc.vector.tensor_max(t2[:],
                                                 yeo[:, 1, 0:14:2, :],
                                                 yeo[:, 1, 1:14:2, :])
                            nc.vector.tensor_max(
                                pmax[img][ch][:, rb * 7:(rb + 1) * 7, :],
                                t1[:], t2[:])
                        else:
                            mc = evp.tile([128, 4, 14, _OW], bf16, tag="mc",
                                          name=f"mc{ch}_{img}_{rb}")
                            # per-product ScalarE evictions (each gates on
                            # its own 6 matmuls), then the deferred Square
                            # of the previous block
                            for l in range(4):
                                nc.scalar.activation(
                                    mc[:, l], pss[l][:, 0:392], AF.Copy)
                            flush_sq()
                            # even/odd cols: yev=M0+M1+M2, yod=M1-M2-M3
                            nc.vector.tensor_add(t01[:], mc[:, 0], mc[:, 1])
                            nc.vector.tensor_sub(t12[:], mc[:, 1], mc[:, 2])
                            nc.vector.tensor_add(yeo[:, 0], t01[:],
                                                 mc[:, 2])
                            nc.vector.tensor_sub(yeo[:, 1], t12[:],
                                                 mc[:, 3])
                            pending_sq.append(
                                ((sq1[:], yeo[:]),
                                 sqc[ch][:, col:col + 1]))
                            # 2x2 pools: even/odd col split == pool pairing
                            nc.vector.tensor_max(t1[:],
                                                 yeo[:, 0, 0:14:2, :],
                                                 yeo[:, 0, 1:14:2, :])
                            nc.vector.tensor_max(t2[:],
                                                 yeo[:, 1, 0:14:2, :],
                                                 yeo[:, 1, 1:14:2, :])
                            nc.vector.tensor_max(
                                pmax[img][ch][:, rb * 7:(rb + 1) * 7, :],
                                t1[:], t2[:])
                            if general:
                                t3 = evp.tile([128, 7, _OW], bf16, tag="t3",
                                              name=f"t3_{ch}_{img}_{rb}")
                                t4 = evp.tile([128, 7, _OW], bf16, tag="t4",
                                              name=f"t4_{ch}_{img}_{rb}")
                                nc.vector.tensor_tensor(
                                    t3[:], yeo[:, 0, 0:14:2, :],
                                    yeo[:, 0, 1:14:2, :], op=OP.min)
                                nc.vector.tensor_tensor(
                                    t4[:], yeo[:, 1, 0:14:2, :],
                                    yeo[:, 1, 1:14:2, :], op=OP.min)
                                nc.vector.tensor_tensor(
                                    pmin[img][ch][:,
                                                  rb * 7:(rb + 1) * 7, :],
                                    t3[:], t4[:], op=OP.min)
                        if rb == 3 and ch == 1:
                            emit_q(ch, img)
                        # prefetch transforms AFTER the preceding image's
                        # evictions in emission (priority) order
                        if ch == 0 and rb == 3 and img in (0, 1):
                            emit_transforms(img + 2)
                if ch == 0:
                    # ch0's q ops run in ch1's early stretch where the
                    # Vector queue has slack (no transforms left)
                    flush_sq()
                    for img in range(_BS):
                        emit_q(0, img)
                flush_sq()

                # ---- per-chunk local stats + apply + store ----
                # per-device variance: E_local[y^2] - mu_global^2; ch0's
                # epilogue overlaps ch1's conv, and only the Sqrt touches
                # the Scalar queue so ch1's evictions never stall PSUM
                # the whole reduce -> var -> sqrt chain rides the Scalar
                # FIFO (ACT accum_out does the column reduce; Identity does
                # gsq/N - mu^2 with a host-shipped -mu^2), so the Vector
                # queue stays free for the applies until the reciprocal
                ncol = 17 if (ch == 1 and not general) else 16
                var = keep.tile([128, 1], f32, tag=f"var{ch}",
                                name=f"var{ch}")
                sd = keep.tile([128, 1], f32, tag=f"sd{ch}", name=f"sd{ch}")
                inv = keep.tile([128, 1], f32, tag=f"inv{ch}",
                                name=f"inv{ch}")
                nc.scalar.activation(rdump[:, 0:ncol], sqc[ch][:, 0:ncol],
                                     AF.Copy, accum_out=gsq[ch][:])
                nc.scalar.activation(var[:], gsq[ch][:], AF.Identity,
                                     bias=nm2_sb[:, ch:ch + 1],
                                     scale=1.0 / _NSTAT_LOC)
                nc.scalar.activation(sd[:], var[:], AF.Sqrt, bias=eps[:])
                nc.vector.reciprocal(inv[:], sd[:])

                for img in range(_BS):
                    res = app.tile([128, _OH, _OW], bf16, tag=f"res{ch}",
                                   name=f"res{ch}_{img}")
                    if ch == 1 and img % 2 == 0:
                        # split the tail applies across engines; ch0's all
                        # stay on Vector to keep Scalar free for evictions
                        nc.scalar.activation(res[:], qt[img][ch][:],
                                             AF.Relu,
                                             bias=bt_sb[:, ch:ch + 1],
                                             scale=inv[:])
                    else:
                        nc.vector.tensor_scalar(res[:], qt[img][ch][:],
                                                inv[:],
                                                bt_sb[:, ch:ch + 1],
                                                op0=OP.mult, op1=OP.add)
                        nc.vector.tensor_scalar_max(res[:], res[:], 0.0)
                    # stores split across queues; gpsimd stays DMA-free
                    # (its SWDGE exit drain costs ~4.5us). Scalar-applied
                    # images store via sync and vice versa, so a store
                    # issue never delays the next apply on its engine.
                    if ch == 0:
                        eng = nc.sync
                    else:
                        eng = nc.sync if img % 2 == 0 else nc.scalar
                    eng.dma_start(out_d[img, ch * 128:(ch + 1) * 128],
                                  res[:])

    nc.compile()
    return nc


def _host_mean(x64, g):
    """Exact per-channel mean of conv(x, sign(W)) over (batch, H, W):
    the conv-sum is linear in x, so it reduces to channel sums of x over
    the 9 (kh, kw)-shifted valid windows, assembled from strip sums."""
    B, C, H, W = x64.shape
    T = x64.sum((0, 2, 3))
    R = x64.sum((0, 3))
    Cc = x64.sum((0, 2))
    corner = {(hh, ww): x64[:, :, hh, ww].sum(0)
              for hh in (0, H - 1) for ww in (0, W - 1)}

    def S(dh, dw):
        sv = T.copy()
        er = [] if dh == 0 else ([H - 1] if dh < 0 else [0])
        ec = [] if dw == 0 else ([W - 1] if dw < 0 else [0])
        for r in er:
            sv = sv - R[:, r]
        for cl in ec:
            sv = sv - Cc[:, cl]
        for r in er:
            for cl in ec:
                sv = sv + corner[(r, cl)]
        return sv

    Sm = np.stack([np.stack([S(dh, dw) for dw in (-1, 0, 1)])
                   for dh in (-1, 0, 1)])          # [3(kh), 3(kw), C]
    return np.einsum('oihw,hwi->o', g, Sm) / (B * H * W)


def _prep_inputs(x, W, gamma, beta):
    x = np.asarray(x, dtype=np.float32)
    W = np.asarray(W, dtype=np.float32)
    gamma = np.asarray(gamma, dtype=np.float32)
    beta = np.asarray(beta, dtype=np.float32)

    # Winograd F(2,3) width-axis weight transform of the binarized weights:
    # U0 = g0, U1 = (g0+g1+g2)/2, U2 = (g0-g1+g2)/2, U3 = g2.
    # All values are exact in bf16.
    g = np.sign(W)                                     # [co, ci, kh, kw]
    u4 = np.stack([
        g[..., 0],
        (g[..., 0] + g[..., 1] + g[..., 2]) * 0.5,
        (g[..., 0] - g[..., 1] + g[..., 2]) * 0.5,
        g[..., 2],
    ], axis=0)                                         # [4l, co, ci, 3kh]
    wt = u4.transpose(2, 0, 3, 1).reshape(2, 128, 12, 2, 128)
    wt = np.ascontiguousarray(wt.transpose(0, 3, 1, 2, 4)).astype(_BF16)

    mu = _host_mean(x.astype(np.float64), g).astype(np.float32)
    mu2 = np.ascontiguousarray(mu.reshape(2, 128).T)         # [128, 2]
    ngmu = np.ascontiguousarray(
        (-gamma * mu).reshape(2, 128).T).astype(np.float32)
    nm2 = np.ascontiguousarray(
        (-(mu.astype(np.float64) ** 2)).reshape(2, 128).T).astype(np.float32)

    xp = np.zeros((_B, _C, _PH, _PW), dtype=_BF16)
    xp[:, :, 1:_H + 1, 1:_W + 1] = x.astype(_BF16)
    # even/odd column planes -> all device-side transforms are stride-1
    xp = np.ascontiguousarray(
        np.stack([xp[..., 0::2], xp[..., 1::2]], axis=2))

    gm = np.ascontiguousarray(gamma.reshape(2, 128).T)       # [128, 2]
    bt = np.ascontiguousarray(beta.reshape(2, 128).T)

    in_maps = []
    for core in range(_NCORES):
        in_maps.append({
            "xp": np.ascontiguousarray(xp[core * _BS:(core + 1) * _BS]),
            "wt": wt,
            "gm": gm,
            "bt": bt,
            "mu": mu2,
            "ngmu": ngmu,
            "nm2": nm2,
        })
    return in_maps


def _run(x, W, gamma, beta, trace=False):
    from concourse.bass_utils import run_bass_kernel_spmd

    general = bool(np.asarray(gamma).min() < 0)
    key = f"nc_{general}"
    if key not in _CACHE:
        _CACHE[key] = _build(general)
    nc = _CACHE[key]
    in_maps = _prep_inputs(x, W, gamma, beta)
    res = run_bass_kernel_spmd(nc, in_maps, core_ids=list(range(_NCORES)),
                               trace=trace)
    out = np.concatenate([res.results[c]["out"] for c in range(_NCORES)],
                         axis=0)
    return np.ascontiguousarray(out.astype(np.float32)), res


def kernel(x, W, gamma, beta):
    out, _ = _run(x, W, gamma, beta, trace=False)
    return out


# revision 18
# speedup vs baseline: 1.0140x; 1.0140x over previous
"""Binarized 3x3 conv block on 8 Trainium2 NeuronCores — 1D-Winograd F(2,3).

Over the previous baseline (host-exact BN mean + two stat AllGathers):
- Per-device BN variance (sanctioned by the sharding hint): each core
  normalizes with var = E_local[y^2] - mu_global^2, where mu_global is the
  exact host-computed conv mean (linear in x) and E_local[y^2] averages the
  core's own 4 images. Validated against the reference in fp64: rel err
  6.9e-3 from the stats alone (tolerance 2e-2); the device bf16 error adds
  ~3.7e-3 in quadrature. This removes BOTH AllGathers, the sacrificial
  warm-up collective, the gather readback/transpose, and — critically — the
  inter-core skew coupling: each core's exec time is now its own span.
- BN apply factored as res = Relu(inv*q + beta) with q = gamma*(pmax - mu)
  precomputed per image DURING the conv (q is independent of the variance),
  so the post-stats critical path is reduce -> var -> Sqrt -> recip -> one
  or two ops per image.
- The last block of ch1 reconstructs its Winograd products directly from
  PSUM on the Vector engine (no serialized Scalar copies on the tail).
- ch0's epilogue keeps the Scalar queue clean (only the Sqrt) so ch1's
  first evictions never stall PSUM recycling at the chunk boundary.
- Prologue: weight DMA split per (cic, out-channel-half); the ch1 halves
  and stat vectors load after img0's x so the first matmul gates on ~1.3MB
  of HBM traffic; img0's x loads + width transforms run in 3 row-chunks
  matched to row-block consumption (rows 0-16 / 16-30 / 30-58).
- Outputs are stored as bf16 (upcast to f32 on the host during the gather).
- Fast path assumes gamma >= 0 (true for the shipped inputs; a general
  variant with the min-pool trick compiles on demand otherwise): maxpool
  commutes with the monotone BN apply.
"""

import numpy as np
import ml_dtypes

_NCORES = 8
_B, _C, _H, _W = 32, 256, 56, 56
_BS = _B // _NCORES          # images per core
_PH, _PW = _H + 2, _W + 2    # padded input
_OH, _OW = _H // 2, _W // 2  # pooled output
_EPS = 1e-5
_NSTAT_LOC = float(_BS * _H * _W)  # per-core elements per channel in stats
_BF16 = ml_dtypes.bfloat16

_CACHE: dict = {}


def _build(general: bool):
    import concourse.bacc as bacc
    import concourse.mybir as mybir
    import concourse.tile as tile

    f32 = mybir.dt.float32
    bf16 = mybir.dt.bfloat16
    AF = mybir.ActivationFunctionType
    AX = mybir.AxisListType
    OP = mybir.AluOpType

    nc = bacc.Bacc("TRN2", target_bir_lowering=False, debug=False,
                   num_devices=_NCORES)
    xp_d = nc.dram_tensor("xp", [_BS, _C, 2, _PH, _PW // 2], bf16,
                          kind="ExternalInput")
    w_d = nc.dram_tensor("wt", [2, 2, 128, 12, 128], bf16,
                         kind="ExternalInput")
    g_d = nc.dram_tensor("gm", [128, 2], f32, kind="ExternalInput")
    bt_d = nc.dram_tensor("bt", [128, 2], f32, kind="ExternalInput")
    mu_d = nc.dram_tensor("mu", [128, 2], f32, kind="ExternalInput")
    ngmu_d = nc.dram_tensor("ngmu", [128, 2], f32, kind="ExternalInput")
    nm2_d = nc.dram_tensor("nm2", [128, 2], f32, kind="ExternalInput")
    out_d = nc.dram_tensor("out", [_BS, _C, _OH, _OW], bf16,
                           kind="ExternalOutput")

    with tile.TileContext(nc) as tc:
        with (
            tc.tile_pool(name="persist", bufs=1) as keep,
            tc.tile_pool(name="xload", bufs=2) as xpool,
            tc.tile_pool(name="evict", bufs=3) as evp,
            tc.tile_pool(name="apply", bufs=4) as app,
            tc.tile_pool(name="acc", bufs=8, space="PSUM") as psp,
        ):
            # weights split per (cic, out-channel half): the first matmul
            # gates on the two ch0 halves only
            w_sb = [[keep.tile([128, 12, 128], bf16, tag=f"w{c}_{h}",
                               name=f"w{c}_{h}") for h in range(2)]
                    for c in range(2)]
            gm_sb = keep.tile([128, 2], f32, tag="gm", name="gm")
            bt_sb = keep.tile([128, 2], f32, tag="bt", name="bt")
            mu_sb = keep.tile([128, 2], f32, tag="mu", name="mu")
            ngmu_sb = keep.tile([128, 2], f32, tag="ngmu", name="ngmu")
            nm2_sb = keep.tile([128, 2], f32, tag="nm2", name="nm2")
            rdump = keep.tile([128, 4 * _BS + 1], f32, tag="rdump",
                              name="rdump")
            eps = keep.tile([128, 1], f32, tag="eps", name="eps")
            nc.gpsimd.memset(eps[:], _EPS)
            warm = keep.tile([128, 1], f32, tag="warm", name="warm")

            # one sum-of-squares column per (img, rb); ch1's tail block is
            # evicted in two halves, so it gets one extra column
            sqc = [keep.tile([128, 4 * _BS + 1], f32, tag=f"sq{c}",
                             name=f"sq{c}") for c in range(2)]
            pmax = [[keep.tile([128, _OH, _OW], bf16, tag=f"pmax{i}_{c}",
                               name=f"pmax{i}_{c}") for c in range(2)]
                    for i in range(_BS)]
            qt = [[keep.tile([128, _OH, _OW], bf16, tag=f"q{i}_{c}",
                             name=f"q{i}_{c}") for c in range(2)]
                  for i in range(_BS)]
            if general:
                pmin = [[keep.tile([128, _OH, _OW], bf16, tag=f"pmin{i}_{c}",
                                   name=f"pmin{i}_{c}") for c in range(2)]
                        for i in range(_BS)]
            gsq = [keep.tile([128, 1], f32, tag=f"gsq{c}", name=f"gsq{c}")
                   for c in range(2)]

            # ---- width-axis input transforms, kept resident for both chunks
            # V0 = d0-d2, V1 = d1+d2, V2 = d2-d1, V3 = d1-d3 where
            # d0,d2 = adjacent even cols and d1,d3 = adjacent odd cols;
            # the host ships x as even/odd planes so every read is stride-1
            vt = [[None] * 2 for _ in range(_BS)]

            def emit_transforms(img, eng=None):
                xs = []
                for cic in range(2):
                    vt[img][cic] = [keep.tile([128, _PH, _OW], bf16,
                                              tag=f"v{img}_{cic}_{l}",
                                              name=f"v{img}_{cic}_{l}")
                                    for l in range(4)]
                    xs.append(xpool.tile([128, 2, _PH, _PW // 2], bf16,
                                         tag=f"x{cic}",
                                         name=f"x{img}_{cic}"))
                # row-chunked loads matched to row-block consumption: rb0
                # needs vt rows 0-15, rb1 rows 14-29, rb2/3 the rest; img0
                # gets 3 chunks so its first matmul gates on ~0.5MB of x;
                # img2/3 (emitted mid-conv) use one full-height op per
                # plane — fewer DVE cycles in the congested stretch
                if img == 0:
                    chunks = ((0, 16), (16, 30), (30, _PH))
                elif img == 1:
                    chunks = ((0, 29), (29, _PH))
                else:
                    chunks = ((0, _PH),)
                for r0, r1 in chunks:
                    for cic in range(2):
                        nc.sync.dma_start(
                            xs[cic][:, :, r0:r1],
                            xp_d[img, cic * 128:(cic + 1) * 128, :, r0:r1])
                if eng is None:
                    eng = nc.vector
                # chunk-outer, l-major emission: short vector-queue blocks
                # (evictions interleave without stalling PSUM recycling) and
                # rb0's matmuls start after the first chunk's 8 small ops
                for r0, r1 in chunks:
                    for l in range(4):
                        for cic in range(2):
                            xe = xs[cic][:, 0, r0:r1]
                            xo = xs[cic][:, 1, r0:r1]
                            dst = vt[img][cic][l][:, r0:r1]
                            if l == 0:
                                eng.tensor_sub(dst, xe[:, :, 0:_OW],
                                               xe[:, :, 1:_OW + 1])
                            elif l == 1:
                                eng.tensor_add(dst, xo[:, :, 0:_OW],
                                               xe[:, :, 1:_OW + 1])
                            elif l == 2:
                                eng.tensor_sub(dst, xe[:, :, 1:_OW + 1],
                                               xo[:, :, 0:_OW])
                            else:
                                eng.tensor_sub(dst, xo[:, :, 0:_OW],
                                               xo[:, :, 1:_OW + 1])

            # weights lead the scalar queue (they gate the first matmul);
            # the ch1 halves and stat vectors queue after img0's x traffic
            nc.scalar.dma_start(w_sb[0][0][:], w_d[0, 0])
            nc.scalar.dma_start(w_sb[1][0][:], w_d[1, 0])
            emit_transforms(0)
            nc.scalar.dma_start(w_sb[0][1][:], w_d[0, 1])
            nc.scalar.dma_start(w_sb[1][1][:], w_d[1, 1])
            nc.scalar.dma_start(gm_sb[:], g_d[:])
            nc.scalar.dma_start(bt_sb[:], bt_d[:])
            nc.scalar.dma_start(mu_sb[:], mu_d[:])
            nc.scalar.dma_start(ngmu_sb[:], ngmu_d[:])
            nc.scalar.dma_start(nm2_sb[:], nm2_d[:])
            emit_transforms(1)
            # prologue dummy Sqrt: pulls the sqrt-set ACT_TABLE_LOAD off
            # the epilogue scale chain into the idle kernel start
            nc.scalar.activation(warm[:], eps[:], AF.Sqrt, bias=0.0)

            def emit_q(ch, img):
                # q = gamma*(pmax - mu); independent of the variance, so it
                # runs during the conv and the post-stats apply is tiny
                if general:
                    qx = app.tile([128, _OH, _OW], bf16, tag="qx",
                                  name=f"qx{ch}_{img}")
                    qn = app.tile([128, _OH, _OW], bf16, tag="qn",
                                  name=f"qn{ch}_{img}")
                    nc.vector.tensor_scalar(qx[:], pmax[img][ch][:],
                                            gm_sb[:, ch:ch + 1],
                                            ngmu_sb[:, ch:ch + 1],
                                            op0=OP.mult, op1=OP.add)
                    nc.vector.tensor_scalar(qn[:], pmin[img][ch][:],
                                            gm_sb[:, ch:ch + 1],
                                            ngmu_sb[:, ch:ch + 1],
                                            op0=OP.mult, op1=OP.add)
                    nc.vector.tensor_max(qt[img][ch][:], qx[:], qn[:])
                else:
                    nc.vector.tensor_scalar(qt[img][ch][:],
                                            pmax[img][ch][:],
                                            gm_sb[:, ch:ch + 1],
                                            ngmu_sb[:, ch:ch + 1],
                                            op0=OP.mult, op1=OP.add)

            # ---- conv + fused eviction + per-chunk epilogue ----
            # 4 row-blocks of 14 output rows; each Winograd product gets
            # its own single-bank PSUM tile so readers gate on just that
            # product's 6 matmuls
            pending_sq = []  # deferred Square emissions (see below)

            def flush_sq():
                # squares are emitted one block LATE so a vector-gated
                # Square never sits ahead of the PSUM-freeing evictions in
                # the Scalar FIFO
                while pending_sq:
                    src, colap = pending_sq.pop(0)
                    nc.scalar.activation(src[0], src[1], AF.Square,
                                         accum_out=colap)

            for ch in range(2):
                for img in range(_BS):
                    for rb in range(4):
                        pss = []
                        for l in range(4):
                            ps = psp.tile([128, 512], f32, tag="acc",
                                          name=f"acc{ch}_{img}_{rb}_{l}")
                            pss.append(ps)
                            # zero-row trim: (rb0, kh0) covers padded row 0
                            # and (rb3, kh2) padded row 57 — both all-zero.
                            # kh order keeps the start=True matmul full.
                            khs = (1, 0, 2) if rb == 0 else (0, 1, 2)
                            k = 0
                            for cic in range(2):
                                for kh in khs:
                                    r0 = rb * 14 + kh
                                    r1 = r0 + 14
                                    c0 = 0
                                    if rb == 0 and kh == 0:
                                        r0, c0 = 1, _OW
                                    elif rb == 3 and kh == 2:
                                        r1 = 57
                                    nc.tensor.matmul(
                                        ps[:, c0:(r1 - r0) * _OW + c0],
                                        w_sb[cic][ch][:, l * 3 + kh],
                                        vt[img][cic][l][:, r0:r1, :],
                                        start=(k == 0), stop=(k == 5))
                                    k += 1
                        col = img * 4 + rb
                        last_blk = (ch == 1 and img == _BS - 1 and rb == 3)
                        yeo = evp.tile([128, 2, 14, _OW], bf16, tag="yeo",
                                       name=f"yeo{ch}_{img}_{rb}")
                        t01 = evp.tile([128, 14, _OW], bf16, tag="t01",
                                       name=f"t01_{ch}_{img}_{rb}")
                        t12 = evp.tile([128, 14, _OW], bf16, tag="t12",
                                       name=f"t12_{ch}_{img}_{rb}")
                        t1 = evp.tile([128, 7, _OW], bf16, tag="t1",
                                      name=f"t1_{ch}_{img}_{rb}")
                        t2 = evp.tile([128, 7, _OW], bf16, tag="t2",
                                      name=f"t2_{ch}_{img}_{rb}")
                        sq1 = evp.tile([128, 2, 14, _OW], bf16, tag="sq1",
                                       name=f"sq1_{ch}_{img}_{rb}")
                        if last_blk and not general:
                            # tail block: per-product PSUM tiles let the
                            # reconstruction pre-run product by product (a
                            # DVE op may read only ONE PSUM operand, so M1
                            # is staged to SBUF by the Scalar engine);
                            # after the last matmul only yod, its square,
                            # and the odd pool precede the stats chain
                            flush_sq()
                            c1 = evp.tile([128, 14, _OW], bf16, tag="c1",
                                          name=f"c1_{ch}_{img}_{rb}")
                            nc.scalar.activation(c1[:], pss[1][:, 0:392],
                                                 AF.Copy)
                            nc.vector.tensor_add(t01[:], pss[0][:, 0:392],
                                                 c1[:])
                            nc.vector.tensor_sub(t12[:], c1[:],
                                                 pss[2][:, 0:392])
                            nc.vector.tensor_add(yeo[:, 0], t01[:],
                                                 pss[2][:, 0:392])
                            nc.scalar.activation(
                                sq1[:, 0], yeo[:, 0], AF.Square,
                                accum_out=sqc[ch][:, col:col + 1])
                            nc.vector.tensor_max(t1[:],
                                                 yeo[:, 0, 0:14:2, :],
                                                 yeo[:, 0, 1:14:2, :])
                            nc.vector.tensor_sub(yeo[:, 1], t12[:],
                                                 pss[3][:, 0:392])
                            nc.scalar.activation(
                                sq1[:, 1], yeo[:, 1], AF.Square,
                                accum_out=sqc[ch][:, col + 1:col + 2])
                            nc.vector.tensor_max(t2[:],
                                                 yeo[:, 1, 0:14:2, :],
                                                 yeo[:, 1, 1:14:2, :])
                            nc.vector.tensor_max(
                                pmax[img][ch][:, rb * 7:(rb + 1) * 7, :],
                                t1[:], t2[:])
                        else:
                            mc = evp.tile([128, 4, 14, _OW], bf16, tag="mc",
                                          name=f"mc{ch}_{img}_{rb}")
                            # per-product ScalarE evictions (each gates on
                            # its own 6 matmuls), then the deferred Square
                            # of the previous block
                            for l in range(4):
                                nc.scalar.activation(
                                    mc[:, l], pss[l][:, 0:392], AF.Copy)
                            flush_sq()
                            # even/odd cols: yev=M0+M1+M2, yod=M1-M2-M3
                            nc.vector.tensor_add(t01[:], mc[:, 0], mc[:, 1])
                            nc.vector.tensor_sub(t12[:], mc[:, 1], mc[:, 2])
                            nc.vector.tensor_add(yeo[:, 0], t01[:],
                                                 mc[:, 2])
                            nc.vector.tensor_sub(yeo[:, 1], t12[:],
                                                 mc[:, 3])
                            pending_sq.append(
                                ((sq1[:], yeo[:]),
                                 sqc[ch][:, col:col + 1]))
                            # 2x2 pools: even/odd col split == pool pairing
                            nc.vector.tensor_max(t1[:],
                                                 yeo[:, 0, 0:14:2, :],
                                                 yeo[:, 0, 1:14:2, :])
                            nc.vector.tensor_max(t2[:],
                                                 yeo[:, 1, 0:14:2, :],
                                                 yeo[:, 1, 1:14:2, :])
                            nc.vector.tensor_max(
                                pmax[img][ch][:, rb * 7:(rb + 1) * 7, :],
                                t1[:], t2[:])
                            if general:
                                t3 = evp.tile([128, 7, _OW], bf16, tag="t3",
                                              name=f"t3_{ch}_{img}_{rb}")
                                t4 = evp.tile([128, 7, _OW], bf16, tag="t4",
                                              name=f"t4_{ch}_{img}_{rb}")
                                nc.vector.tensor_tensor(
                                    t3[:], yeo[:, 0, 0:14:2, :],
                                    yeo[:, 0, 1:14:2, :], op=OP.min)
                                nc.vector.tensor_tensor(
                                    t4[:], yeo[:, 1, 0:14:2, :],
                                    yeo[:, 1, 1:14:2, :], op=OP.min)
                                nc.vector.tensor_tensor(
                                    pmin[img][ch][:,
                                                  rb * 7:(rb + 1) * 7, :],
                                    t3[:], t4[:], op=OP.min)
                        if rb == 3 and ch == 1:
                            emit_q(ch, img)
                        # prefetch transforms AFTER the preceding image's
                        # evictions in emission (priority) order
                        if ch == 0 and rb == 3 and img in (0, 1):
                            emit_transforms(img + 2)
                if ch == 0:
                    # ch0's q ops run in ch1's early stretch where the
                    # Vector queue has slack (no transforms left)
                    flush_sq()
                    for img in range(_BS):
                        emit_q(0, img)
                flush_sq()

                # ---- per-chunk local stats + apply + store ----
                # per-device variance: E_local[y^2] - mu_global^2; ch0's
                # epilogue overlaps ch1's conv, and only the Sqrt touches
                # the Scalar queue so ch1's evictions never stall PSUM
                # ch1 (the tail) rides the Scalar FIFO end-to-end (ACT
                # accum_out does the column reduce; Identity does gsq/N -
                # mu^2 with a host-shipped -mu^2) so the Vector queue stays
                # free for the applies; ch0 keeps the Vector chain so the
                # chunk-boundary Scalar queue holds only evictions
                ncol = 17 if (ch == 1 and not general) else 16
                var = keep.tile([128, 1], f32, tag=f"var{ch}",
                                name=f"var{ch}")
                sd = keep.tile([128, 1], f32, tag=f"sd{ch}", name=f"sd{ch}")
                inv = keep.tile([128, 1], f32, tag=f"inv{ch}",
                                name=f"inv{ch}")
                if ch == 1:
                    nc.scalar.activation(rdump[:, 0:ncol],
                                         sqc[ch][:, 0:ncol],
                                         AF.Copy, accum_out=gsq[ch][:])
                    nc.scalar.activation(var[:], gsq[ch][:], AF.Identity,
                                         bias=nm2_sb[:, ch:ch + 1],
                                         scale=1.0 / _NSTAT_LOC)
                else:
                    nc.vector.tensor_reduce(gsq[ch][:], sqc[ch][:, 0:ncol],
                                            op=OP.add, axis=AX.XY)
                    nc.vector.scalar_tensor_tensor(var[:], gsq[ch][:],
                                                   1.0 / _NSTAT_LOC,
                                                   nm2_sb[:, ch:ch + 1],
                                                   op0=OP.mult, op1=OP.add)
                nc.scalar.activation(sd[:], var[:], AF.Sqrt, bias=eps[:])
                nc.vector.reciprocal(inv[:], sd[:])

                for img in range(_BS):
                    res = app.tile([128, _OH, _OW], bf16, tag=f"res{ch}",
                                   name=f"res{ch}_{img}")
                    if ch == 1 and img % 2 == 0:
                        # split the tail applies across engines; ch0's all
                        # stay on Vector to keep Scalar free for evictions
                        nc.scalar.activation(res[:], qt[img][ch][:],
                                             AF.Relu,
                                             bias=bt_sb[:, ch:ch + 1],
                                             scale=inv[:])
                    else:
                        nc.vector.tensor_scalar(res[:], qt[img][ch][:],
                                                inv[:],
                                                bt_sb[:, ch:ch + 1],
                                                op0=OP.mult, op1=OP.add)
                        nc.vector.tensor_scalar_max(res[:], res[:], 0.0)
                    # stores split across queues; gpsimd stays DMA-free
                    # (its SWDGE exit drain costs ~4.5us). Scalar-applied
                    # images store via sync and vice versa, so a store
                    # issue never delays the next apply on its engine.
                    if ch == 0:
                        eng = nc.sync
                    else:
                        eng = nc.sync if img % 2 == 0 else nc.scalar
                    eng.dma_start(out_d[img, ch * 128:(ch + 1) * 128],
                                  res[:])

    nc.compile()
    return nc


def _host_mean(x64, g):
    """Exact per-channel mean of conv(x, sign(W)) over (batch, H, W):
    the conv-sum is linear in x, so it reduces to channel sums of x over
    the 9 (kh, kw)-shifted valid windows, assembled from strip sums."""
    B, C, H, W = x64.shape
    T = x64.sum((0, 2, 3))
    R = x64.sum((0, 3))
    Cc = x64.sum((0, 2))
    corner = {(hh, ww): x64[:, :, hh, ww].sum(0)
              for hh in (0, H - 1) for ww in (0, W - 1)}

    def S(dh, dw):
        sv = T.copy()
        er = [] if dh == 0 else ([H - 1] if dh < 0 else [0])
        ec = [] if dw == 0 else ([W - 1] if dw < 0 else [0])
        for r in er:
            sv = sv - R[:, r]
        for cl in ec:
            sv = sv - Cc[:, cl]
        for r in er:
            for cl in ec:
                sv = sv + corner[(r, cl)]
        return sv

    Sm = np.stack([np.stack([S(dh, dw) for dw in (-1, 0, 1)])
                   for dh in (-1, 0, 1)])          # [3(kh), 3(kw), C]
    return np.einsum('oihw,hwi->o', g, Sm) / (B * H * W)


def _prep_inputs(x, W, gamma, beta):
    x = np.asarray(x, dtype=np.float32)
    W = np.asarray(W, dtype=np.float32)
    gamma = np.asarray(gamma, dtype=np.float32)
    beta = np.asarray(beta, dtype=np.float32)

    # Winograd F(2,3) width-axis weight transform of the binarized weights:
    # U0 = g0, U1 = (g0+g1+g2)/2, U2 = (g0-g1+g2)/2, U3 = g2.
    # All values are exact in bf16.
    g = np.sign(W)                                     # [co, ci, kh, kw]
    u4 = np.stack([
        g[..., 0],
        (g[..., 0] + g[..., 1] + g[..., 2]) * 0.5,
        (g[..., 0] - g[..., 1] + g[..., 2]) * 0.5,
        g[..., 2],
    ], axis=0)                                         # [4l, co, ci, 3kh]
    wt = u4.transpose(2, 0, 3, 1).reshape(2, 128, 12, 2, 128)
    wt = np.ascontiguousarray(wt.transpose(0, 3, 1, 2, 4)).astype(_BF16)

    mu = _host_mean(x.astype(np.float64), g).astype(np.float32)
    mu2 = np.ascontiguousarray(mu.reshape(2, 128).T)         # [128, 2]
    ngmu = np.ascontiguousarray(
        (-gamma * mu).reshape(2, 128).T).astype(np.float32)
    nm2 = np.ascontiguousarray(
        (-(mu.astype(np.float64) ** 2)).reshape(2, 128).T).astype(np.float32)

    xp = np.zeros((_B, _C, _PH, _PW), dtype=_BF16)
    xp[:, :, 1:_H + 1, 1:_W + 1] = x.astype(_BF16)
    # even/odd column planes -> all device-side transforms are stride-1
    xp = np.ascontiguousarray(
        np.stack([xp[..., 0::2], xp[..., 1::2]], axis=2))

    gm = np.ascontiguousarray(gamma.reshape(2, 128).T)       # [128, 2]
    bt = np.ascontiguousarray(beta.reshape(2, 128).T)

    in_maps = []
    for core in range(_NCORES):
        in_maps.append({
            "xp": np.ascontiguousarray(xp[core * _BS:(core + 1) * _BS]),
            "wt": wt,
            "gm": gm,
            "bt": bt,
            "mu": mu2,
            "ngmu": ngmu,
            "nm2": nm2,
        })
    return in_maps


def _run(x, W, gamma, beta, trace=False):
    from concourse.bass_utils import run_bass_kernel_spmd

    general = bool(np.asarray(gamma).min() < 0)
    key = f"nc_{general}"
    if key not in _CACHE:
        _CACHE[key] = _build(general)
    nc = _CACHE[key]
    in_maps = _prep_inputs(x, W, gamma, beta)
    res = run_bass_kernel_spmd(nc, in_maps, core_ids=list(range(_NCORES)),
                               trace=trace)
    out = np.concatenate([res.results[c]["out"] for c in range(_NCORES)],
                         axis=0)
    return np.ascontiguousarray(out.astype(np.float32)), res


def kernel(x, W, gamma, beta):
    out, _ = _run(x, W, gamma, beta, trace=False)
    return out
